# revision 1
# baseline (speedup 1.0000x reference)
"""Bass/Trainium2 kernel for nn_KernelEdges (gnn_message_passing).

Computes A = exp((g_i + g_j - 2*Xf@Xf.T)/sigma^2) with zeroed diagonal,
broadcast to all B batch slots, where Xf = X.transpose(1,0,2).reshape(N, B*d).

Sharding: rows of the NxN pairwise matrix are split across 8 NeuronCores
(256 rows each).  Each core receives the full transposed operand
XT = Xf.T [B*d, N] (host-prepared, 4 MB), its own column-slice as the
stationary matmul operand, and writes its [B, N/8, N] output slice.

Per-core device work:
  psum[mt,nb] = sum_q XT_q[:, m_slice].T @ XT_q[:, n_block]     (Gram matrix)
              + (-1/2*ones).T @ g_row[n_block]                  (rank-1: -g_j/2)
  A = exp(-2/sigma^2 * psum + g_i/sigma^2)                      (ACT, bias per row)
  DMA A tile to the 8 batch slots of the output.

The diagonal is zeroed on the host (16K elements) after the gather.
"""

import numpy as np

B, N, D = 8, 2048, 64
NCORES = 8
R = N // NCORES          # 256 rows per core
KD = B * D               # 512 contraction dim
NB = 512                 # n-block (one PSUM bank of fp32)
NNB = N // NB            # 4 n-blocks
NMT = R // 128           # 2 m-tiles per core
NQ = KD // 128           # 4 k-tiles

# matmul operand dtype: "f32r" (full-rate fp32 mode, ~4e-4 rel err) or
# "bf16" (half the input bytes + faster PE, ~2e-3 rel err)
MM_MODE = "f32r"


def _build_program(inv_s2):
    import concourse.bass as bass
    import concourse.tile as tile
    from concourse import bacc, mybir

    f32 = mybir.dt.float32
    mm_dt = mybir.dt.bfloat16 if MM_MODE == "bf16" else mybir.dt.float32r

    nc = bacc.Bacc(
        "TRN2", target_bir_lowering=False, debug=False, num_devices=NCORES
    )

    GK = 2 if MM_MODE == "bf16" else 1  # g carried as hi+lo rows in bf16

    xt_d = nc.dram_tensor("xt", [KD, N], mm_dt, kind="ExternalInput").ap()
    lhst_d = nc.dram_tensor("lhst", [KD, R], mm_dt, kind="ExternalInput").ap()
    grow_d = nc.dram_tensor("grow", [GK, N], mm_dt, kind="ExternalInput").ap()
    bias_d = nc.dram_tensor("bias", [128, NMT], f32, kind="ExternalInput").ap()
    out_d = nc.dram_tensor("out", [B, R, N], f32, kind="ExternalOutput").ap()

    with tile.TileContext(nc) as tc:
        with (
            tc.tile_pool(name="persist", bufs=1) as persist,
            tc.tile_pool(name="apool", bufs=1) as apool,
            tc.tile_pool(name="psum", bufs=1, space="PSUM") as pspool,
        ):
            # ---- loads ----
            # all input DMAs go on the scalar (ACT) HWDGE ring so the sync
            # ring is dedicated to output DMAs.
            # tiny tensors first: the rank-1 g_j matmuls depend only on
            # these, so they start during the xt load and warm the PE
            grow_sb = persist.tile([GK, N], mm_dt, name="grow")
            nc.scalar.dma_start(grow_sb[:], grow_d[:])

            bias_sb = persist.tile([128, NMT], f32, name="bias")
            nc.scalar.dma_start(bias_sb[:], bias_d[:])

            neg_half = persist.tile([GK, 128], mm_dt, name="neg_half")
            # -0.5 bit pattern; direct float memset into f32r fails ISA check
            if MM_MODE == "bf16":
                nc.gpsimd.memset(
                    neg_half[:].bitcast(mybir.dt.uint16), 0xBF00
                )
            else:
                nc.gpsimd.memset(
                    neg_half[:].bitcast(mybir.dt.uint32), 0xBF000000
                )

            lhs_sb = persist.tile([128, NQ * R], mm_dt, name="lhs")
            nc.scalar.dma_start(
                lhs_sb[:].rearrange("p (q m) -> p q m", q=NQ),
                lhst_d.rearrange("(q p) m -> p q m", p=128),
            )

            # xt tiles; the last one split into n-block pieces so the
            # trailing piece (which gates the final matmul batch) is small
            xt_sb = []
            for q in range(NQ):
                t = persist.tile([128, N], mm_dt, name=f"xt{q}")
                if q < NQ - 1:
                    nc.scalar.dma_start(t[:], xt_d[q * 128:(q + 1) * 128, :])
                else:
                    for nb in range(NNB):
                        sl = slice(nb * NB, (nb + 1) * NB)
                        nc.scalar.dma_start(
                            t[:, sl], xt_d[q * 128:(q + 1) * 128, sl]
                        )
                xt_sb.append(t)

            # ---- compute + store ----
            # all 8 accumulation chains live in the 8 PSUM banks at once;
            # chain order: rank-1 (g_j) first, then k-tiles q0..q3 as each
            # xt_q lands, so the PE overlaps the input DMA
            ps = {}
            for mt in range(NMT):
                for nb in range(NNB):
                    ps[mt, nb] = pspool.tile(
                        [128, NB], f32, name=f"ps{mt}{nb}"
                    )
                    nc.tensor.matmul(
                        ps[mt, nb][:],
                        neg_half[:],
                        grow_sb[:, nb * NB:(nb + 1) * NB],
                        start=True,
                        stop=False,
                    )
            a_sb = {
                mt: apool.tile([128, N], f32, name=f"a{mt}")
                for mt in range(NMT)
            }
            for q in range(NQ):
                last = q == NQ - 1
                # last k-tile arrives in nb pieces: nb-major order so each
                # piece unblocks its matmuls immediately (PE is in-order)
                order = (
                    [(mt, nb) for nb in range(NNB) for mt in range(NMT)]
                    if last
                    else [
                        (mt, nb)
                        for h in range(2)
                        for mt in range(NMT)
                        for nb in range(2 * h, 2 * h + 2)
                    ]
                )
                for mt, nb in order:
                    m0 = q * R + mt * 128
                    nc.tensor.matmul(
                        ps[mt, nb][:],
                        lhs_sb[:, m0:m0 + 128],
                        xt_sb[q][:, nb * NB:(nb + 1) * NB],
                        start=False,
                        stop=last,
                    )
            # ACTs in mt-major order so mt0's output DMA launches as soon
            # as its four n-blocks are done (Scalar executes in FIFO order)
            for mt in range(NMT):
                for nb in range(NNB):
                    nc.scalar.activation(
                        a_sb[mt][:, nb * NB:(nb + 1) * NB],
                        ps[mt, nb][:],
                        mybir.ActivationFunctionType.Exp,
                        bias=bias_sb[:, mt:mt + 1],
                        scale=-2.0 * inv_s2,
                    )
            # one DMA per m-tile replicates [128, 2048] into all 8 batch
            # slots: 8 KB contiguous runs in DRAM
            for mt in range(NMT):
                src = a_sb[mt][:].rearrange(
                    "p (o n) -> p o n", o=1
                ).broadcast_to([128, B, N])
                dst = out_d[
                    :, mt * 128:(mt + 1) * 128, :
                ].rearrange("b p n -> p b n")
                nc.sync.dma_start(dst, src)

    nc.compile()
    return nc


def _prepare(X, log_sigma):
    """Host prep: returns (inv_s2, in_maps) for run_bass_kernel_spmd."""
    X = np.ascontiguousarray(X, dtype=np.float32)
    assert X.shape == (B, N, D), X.shape

    sigma = float(np.exp(np.float32(log_sigma)))
    inv_s2 = 1.0 / (sigma * sigma)

    # XT[b*D+f, n] = X[b, n, f]
    XT = np.ascontiguousarray(X.transpose(0, 2, 1).reshape(KD, N))
    g = np.einsum("kn,kn->n", XT, XT).astype(np.float32)  # [N]
    if MM_MODE == "bf16":
        import ml_dtypes

        XT = np.ascontiguousarray(XT.astype(ml_dtypes.bfloat16))
        g_hi = g.astype(ml_dtypes.bfloat16)
        g_lo = (g - g_hi.astype(np.float32)).astype(ml_dtypes.bfloat16)
        grow_np = np.stack([g_hi, g_lo])  # [2, N]
    else:
        grow_np = g[None, :]

    in_maps = []
    for c in range(NCORES):
        r0 = c * R
        bias_np = np.empty((128, NMT), dtype=np.float32)
        for mt in range(NMT):
            bias_np[:, mt] = g[r0 + mt * 128: r0 + (mt + 1) * 128] * inv_s2
        in_maps.append({
            "xt": XT,
            "lhst": np.ascontiguousarray(XT[:, r0:r0 + R]),
            "grow": grow_np,
            "bias": bias_np,
        })
    return inv_s2, in_maps


def kernel(X, log_sigma):
    from concourse.bass_utils import run_bass_kernel_spmd

    inv_s2, in_maps = _prepare(X, log_sigma)
    nc = _build_program(inv_s2)
    res = run_bass_kernel_spmd(nc, in_maps, list(range(NCORES)))
    out = np.concatenate([res.results[c]["out"] for c in range(NCORES)], axis=1)
    idx = np.arange(N)
    out[:, idx, idx] = 0.0
    return out



# revision 2
# speedup vs baseline: 2.3806x; 2.3806x over previous
"""Bass/Trainium2 kernel for nn_KernelEdges (gnn_message_passing).

Computes A = exp((g_i + g_j - 2*Xf@Xf.T)/sigma^2) with zeroed diagonal,
broadcast to all B batch slots, where Xf = X.transpose(1,0,2).reshape(N, B*d).

Sharding: rows of the NxN pairwise matrix are split across 8 NeuronCores
(256 rows each).  The batch dim of the output is a pure replication of the
same [N, N] matrix, so each core writes only its unique [N/8, N] tile and
the host broadcasts to the B batch slots (as the reference itself does).

Each core receives a column-ROLLED copy of XT = Xf.T [B*d, N] so that its
own 256 columns sit at rolled positions 0..255; the matmul LHS (stationary
operand) is then a fixed slice of the streamed xt tile and no separate
lhst input is needed.  The host un-rolls the output columns after gather.

Per-core device work (per psum chain (mt, nb) of [128, 512]):
  psum = (-1/2*ones).T @ g_row[nb]                       (rank-1: -g_j/2)
       + sum_q xt_q[:, mt*128:+128].T @ xt_q[:, nb]      (Gram matrix)
  A    = exp(-2/sigma^2 * psum + g_i/sigma^2)            (ACT, bias per row)
  DMA A piece [128, 512] to its slot of the [N/8, N] output tile.

Input pieces stream in nb-major order so chain nb finishes (and its ACT +
store start) while pieces for nb+1 are still loading: reads and writes
overlap instead of serializing.

The diagonal is zeroed on the host (2K elements) after the gather.
"""

import numpy as np

B, N, D = 8, 2048, 64
NCORES = 8
R = N // NCORES          # 256 rows per core
KD = B * D               # 512 contraction dim
NB = 512                 # n-block (one PSUM bank of fp32)
NNB = N // NB            # 4 n-blocks
NMT = R // 128           # 2 m-tiles per core
NQ = KD // 128           # 4 k-tiles

# matmul operand dtype: "bf16" (half input bytes + 2x PE rate, ~2e-3 rel
# err) or "f32r" (full-rate fp32 mode, ~4e-4 rel err)
MM_MODE = "bf16"
# store A as bf16 (half the output bytes, ~4e-3 rel err) and upcast on host
OUT_BF16 = True


def _build_program(inv_s2):
    import concourse.bass as bass
    import concourse.tile as tile
    from concourse import bacc, mybir

    f32 = mybir.dt.float32
    mm_dt = mybir.dt.bfloat16 if MM_MODE == "bf16" else mybir.dt.float32r
    out_dt = mybir.dt.bfloat16 if OUT_BF16 else f32

    nc = bacc.Bacc(
        "TRN2", target_bir_lowering=False, debug=False, num_devices=NCORES
    )

    GK = 2 if MM_MODE == "bf16" else 1  # g carried as hi+lo rows in bf16

    # xt pre-tiled on host: piece (nb, q) = rows (nb*NQ+q)*128..+128, fully
    # contiguous in DRAM for max DMA efficiency
    xt_d = nc.dram_tensor(
        "xt", [NNB * NQ * 128, NB], mm_dt, kind="ExternalInput"
    ).ap()
    grow_d = nc.dram_tensor("grow", [GK, N], mm_dt, kind="ExternalInput").ap()
    bias_d = nc.dram_tensor("bias", [128, NMT], f32, kind="ExternalInput").ap()
    # out piece (mt, nb) at rows (mt*NNB+nb)*128..+128, contiguous
    out_d = nc.dram_tensor(
        "out", [NMT * NNB * 128, NB], out_dt, kind="ExternalOutput"
    ).ap()

    with tile.TileContext(nc) as tc:
        with (
            tc.tile_pool(name="persist", bufs=1) as persist,
            tc.tile_pool(name="apool", bufs=1) as apool,
            tc.tile_pool(name="psum", bufs=1, space="PSUM") as pspool,
        ):
            # ---- loads ----
            # all input DMAs go on the scalar (ACT) HWDGE ring so the sync
            # ring is dedicated to output DMAs.
            # tiny tensors first: the rank-1 g_j matmuls depend only on
            # these, so they run during the xt load and warm the PE
            grow_sb = persist.tile([GK, N], mm_dt, name="grow")
            nc.scalar.dma_start(grow_sb[:], grow_d[:])

            bias_sb = persist.tile([128, NMT], f32, name="bias")
            nc.scalar.dma_start(bias_sb[:], bias_d[:])

            neg_half = persist.tile([GK, 128], mm_dt, name="neg_half")
            # -0.5 bit pattern; direct float memset into f32r fails ISA check
            if MM_MODE == "bf16":
                nc.gpsimd.memset(
                    neg_half[:].bitcast(mybir.dt.uint16), 0xBF00
                )
            else:
                nc.gpsimd.memset(
                    neg_half[:].bitcast(mybir.dt.uint32), 0xBF000000
                )

            # xt pieces stream nb-major: all k-tiles of n-block 0 first, so
            # the nb0 psum chains finish (and their ACT + store launch)
            # while nb1..3 pieces are still loading
            xt_sb = [
                persist.tile([128, N], mm_dt, name=f"xt{q}")
                for q in range(NQ)
            ]
            for nb in range(NNB):
                for q in range(NQ):
                    row0 = (nb * NQ + q) * 128
                    nc.scalar.dma_start(
                        xt_sb[q][:, nb * NB:(nb + 1) * NB],
                        xt_d[row0:row0 + 128, :],
                    )

            # ---- compute + store ----
            # all 8 accumulation chains live in the 8 PSUM banks at once;
            # rank-1 (g_j) seeds first (needs only grow), then each chain's
            # k-tiles run back-to-back as its pieces land
            ps = {}
            for nb in range(NNB):
                for mt in range(NMT):
                    ps[mt, nb] = pspool.tile(
                        [128, NB], f32, name=f"ps{mt}{nb}"
                    )
                    nc.tensor.matmul(
                        ps[mt, nb][:],
                        neg_half[:],
                        grow_sb[:, nb * NB:(nb + 1) * NB],
                        start=True,
                        stop=False,
                    )
            a_sb = {
                mt: apool.tile([128, N], out_dt, name=f"a{mt}")
                for mt in range(NMT)
            }
            # matmul order matches piece arrival order (PE is in-order):
            # LHS is the core's own 256 rolled columns, a slice of piece
            # (nb=0, q) which is always already resident
            for nb in range(NNB):
                for q in range(NQ):
                    for mt in range(NMT):
                        nc.tensor.matmul(
                            ps[mt, nb][:],
                            xt_sb[q][:, mt * 128:(mt + 1) * 128],
                            xt_sb[q][:, nb * NB:(nb + 1) * NB],
                            start=False,
                            stop=(q == NQ - 1),
                        )
            # ACT + store chase the chains in their stop order
            for nb in range(NNB):
                for mt in range(NMT):
                    nc.scalar.activation(
                        a_sb[mt][:, nb * NB:(nb + 1) * NB],
                        ps[mt, nb][:],
                        mybir.ActivationFunctionType.Exp,
                        bias=bias_sb[:, mt:mt + 1],
                        scale=-2.0 * inv_s2,
                    )
                    row0 = (mt * NNB + nb) * 128
                    nc.sync.dma_start(
                        out_d[row0:row0 + 128, :],
                        a_sb[mt][:, nb * NB:(nb + 1) * NB],
                    )

    nc.compile()
    return nc


def _prepare(X, log_sigma):
    """Host prep: returns (inv_s2, in_maps) for run_bass_kernel_spmd."""
    import ml_dtypes

    X = np.ascontiguousarray(X, dtype=np.float32)
    assert X.shape == (B, N, D), X.shape

    sigma = float(np.exp(np.float32(log_sigma)))
    inv_s2 = 1.0 / (sigma * sigma)

    # XT[b*D+f, n] = X[b, n, f]
    XT = np.ascontiguousarray(X.transpose(0, 2, 1).reshape(KD, N))
    g = np.einsum("kn,kn->n", XT, XT).astype(np.float32)  # [N]

    mm_np = ml_dtypes.bfloat16 if MM_MODE == "bf16" else np.float32
    XTm = XT.astype(mm_np)

    in_maps = []
    for c in range(NCORES):
        r0 = c * R
        # roll columns so this core's own block is at rolled cols 0..R-1
        Xr = np.roll(XTm, -r0, axis=1)
        # pre-tile: piece (nb, q) contiguous -> [NNB*NQ*128, NB]
        xt_t = np.ascontiguousarray(
            Xr.reshape(NQ, 128, NNB, NB).transpose(2, 0, 1, 3)
        ).reshape(NNB * NQ * 128, NB)

        gr = np.roll(g, -r0)
        if MM_MODE == "bf16":
            g_hi = gr.astype(ml_dtypes.bfloat16)
            g_lo = (gr - g_hi.astype(np.float32)).astype(ml_dtypes.bfloat16)
            grow_np = np.ascontiguousarray(np.stack([g_hi, g_lo]))  # [2, N]
        else:
            grow_np = np.ascontiguousarray(gr[None, :])

        bias_np = np.empty((128, NMT), dtype=np.float32)
        for mt in range(NMT):
            bias_np[:, mt] = g[r0 + mt * 128: r0 + (mt + 1) * 128] * inv_s2
        in_maps.append({
            "xt": xt_t,
            "grow": grow_np,
            "bias": bias_np,
        })
    return inv_s2, in_maps


def kernel(X, log_sigma):
    from concourse.bass_utils import run_bass_kernel_spmd

    inv_s2, in_maps = _prepare(X, log_sigma)
    nc = _build_program(inv_s2)
    res = run_bass_kernel_spmd(nc, in_maps, list(range(NCORES)))

    A = np.empty((N, N), dtype=np.float32)
    for c in range(NCORES):
        r0 = c * R
        t = np.asarray(res.results[c]["out"])
        # un-tile: [NMT*NNB*128, NB] -> [R, N] (still column-rolled)
        t = t.reshape(NMT, NNB, 128, NB).transpose(0, 2, 1, 3).reshape(R, N)
        # un-roll columns back to global positions
        A[r0:r0 + R, :] = np.roll(t.astype(np.float32), r0, axis=1)
    idx = np.arange(N)
    A[idx, idx] = 0.0
    out = np.empty((B, N, N), dtype=np.float32)
    out[:] = A[None, :, :]
    return out
